# revision 5
# baseline (speedup 1.0000x reference)
"""Trainium2 Bass kernel v3 for nn_ColorFeatureExtractor (per-image KMeans).

Data parallel: image b -> core b; 100 Lloyd iterations on-chip; trajectories
streamed back; host picks convergence iteration (same contract as baseline).

v3 vs v2: no gpsimd (it contends with DVE for SBUF ports). fp16 masks and
fp16 channel-major pixel copy; products for all 4 clusters batched into one
2x-rate fp16 TT per channel; sums accumulated on ScalarE ACT (except a few
routed to DVE STT+acc for balance). Counts ride the mask STTs' accumulators
(exact integer sums). Tail on replicated rows.
"""
import os
import sys
import numpy as np

for _p in ("/opt/trn_rl_repo", "/root/.axon_site/_ro/trn_rl_repo"):
    if _p not in sys.path:
        sys.path.append(_p)

K = 5
N = 224 * 224
P = 128
F = N // P             # 392
ITERS = int(os.environ.get("KM_ITERS", "100"))
RTOL, ATOL = 1e-5, 1e-8
OUT_LEN = 500 + 101 * 15

# sums routing: 'S' = ScalarE ACT+acc reading the batched fp16 product,
#               'V' = DVE STT+acc computing product+sum itself (fp16 pixels)
# prod16: batched fp16 products (0 -> fp32 separate TT products, no batching)
_DEF = "sums=SSSSVVSVVSVV;prod16=1"
CFG = os.environ.get("KM_CFG", _DEF)

_CACHE = {}


def _parse_cfg(cfg):
    parts = dict(p.split("=", 1) for p in cfg.split(";") if p)
    sums = parts.get("sums", "S" * 12)
    assert len(sums) == 12 and set(sums) <= set("VS")
    return sums, parts.get("prod16", "1") == "1"


def _build_nc():
    import concourse.bass as bass
    import concourse.mybir as mybir
    from concourse import bacc, tile

    f32 = mybir.dt.float32
    f16 = mybir.dt.float16
    Alu = mybir.AluOpType
    Act = mybir.ActivationFunctionType
    X = mybir.AxisListType.X

    SUMS, PROD16 = _parse_cfg(CFG)
    mdt = f16 if PROD16 else f32

    nc = bacc.Bacc(None, target_bir_lowering=False)
    xp = nc.dram_tensor("xp", [3, N], f32, kind="ExternalInput")
    cbin = nc.dram_tensor("cbin", [1, 20], f32, kind="ExternalInput")
    outv = nc.dram_tensor("outv", [1, OUT_LEN], f32, kind="ExternalOutput")

    with tile.TileContext(nc) as tc:
        with (
            tc.tile_pool(name="persist", bufs=1) as pp,
            tc.tile_pool(name="sco", bufs=2) as sco,
            tc.tile_pool(name="msk", bufs=2) as msk,
            tc.tile_pool(name="jnk", bufs=3) as jnk,
            tc.tile_pool(name="sm", bufs=2) as sm,
            tc.tile_pool(name="ps", bufs=2, space=bass.MemorySpace.PSUM) as ps,
        ):
            # ---- persistent data ----
            pc = pp.tile([P, 3 * F], f32, tag="pc")     # channel-major pixels
            pc16 = pp.tile([P, 3 * F], mdt, tag="pc16")  # product copy
            ones128 = pp.tile([P, 128], f32, tag="ones128")
            onesr = pp.tile([1, 128], f32, tag="onesr")
            tot3 = pp.tile([P, 3], f32, tag="tot3")
            counts_st = pp.tile([1, 500], f32, tag="counts_st")
            cent_st = pp.tile([1, 101 * 15], f32, tag="cent_st")
            cb0 = pp.tile([1, 20], f32, tag="cb0")

            nc.vector.memset(counts_st[:], 0.0)
            nc.vector.memset(cent_st[:], 0.0)
            nc.vector.memset(ones128[:], 1.0)
            nc.vector.memset(onesr[:], 1.0)

            xap = xp[:].rearrange("c (p f) -> c p f", p=P)
            for d in range(3):
                nc.sync.dma_start(out=pc[:, d * F:(d + 1) * F], in_=xap[d])
            nc.sync.dma_start(out=cb0[:], in_=cbin[:])

            px = pc[:, 0:F]
            py = pc[:, F:2 * F]
            pz = pc[:, 2 * F:3 * F]
            for d in range(3):
                nc.vector.tensor_scalar(pc[:, d * F:(d + 1) * F],
                                        pc[:, d * F:(d + 1) * F],
                                        1e-8, None, Alu.add)
            nc.vector.tensor_copy(pc16[:], pc[:])

            # totals per channel from the PRODUCT pixels (consistency for S4)
            totc = pp.tile([P, 3], f32, tag="totc")
            for d in range(3):
                nc.vector.tensor_reduce(totc[:, d:d + 1],
                                        pc16[:, d * F:(d + 1) * F], X, Alu.add)
            tot3_ps = ps.tile([P, 3], f32, tag="tot3ps")
            nc.tensor.matmul(tot3_ps[:], ones128[:], totc[:], start=True, stop=True)
            nc.vector.tensor_copy(tot3[:], tot3_ps[:])

            nc.scalar.copy(cent_st[0:1, 0:15], cb0[0:1, 0:15])
            rep_ps0 = ps.tile([P, 20], f32, tag="repps0")
            nc.tensor.matmul(rep_ps0[:], onesr[:], cb0[:], start=True, stop=True)
            rep = pp.tile([P, 20], f32, tag="rep0")
            nc.vector.tensor_copy(rep[:], rep_ps0[:])

            prev_traj = None
            for t in range(1, ITERS + 1):
                # ---------- phase 1: scores (identical arithmetic to baseline) ----------
                # scores in one contiguous tile so max/masks can batch
                s5 = sco.tile([P, 5 * F], f32, tag="s5")
                s_tiles = [s5[:, k * F:(k + 1) * F] for k in range(5)]
                for k in range(5):
                    u = jnk.tile([P, F], f32, tag=f"u{k}")
                    nc.scalar.activation(
                        u[:], px, Act.Identity,
                        bias=rep[:, 15 + k:16 + k], scale=rep[:, 3 * k:3 * k + 1],
                    )
                    v = jnk.tile([P, F], f32, tag=f"v{k}")
                    nc.vector.scalar_tensor_tensor(
                        v[:], py, rep[:, 3 * k + 1:3 * k + 2], u[:], Alu.mult, Alu.add)
                    nc.vector.scalar_tensor_tensor(
                        s_tiles[k], pz, rep[:, 3 * k + 2:3 * k + 3], v[:], Alu.mult, Alu.add)

                # previous iteration's trajectory snapshots go behind the u's
                # on ScalarE's queue so they don't delay next-iter scores
                if prev_traj is not None:
                    pcn, pcb, pt = prev_traj
                    nc.scalar.copy(counts_st[0:1, 5 * (pt - 1):5 * pt], pcn[0:1, :])
                    nc.scalar.copy(cent_st[0:1, 15 * pt:15 * (pt + 1)], pcb[0:1, 0:15])

                # ---------- phase 2: max (exact under any association) ----------
                mm2 = jnk.tile([P, 2 * F], f32, tag="mm2")
                mC = jnk.tile([P, F], f32, tag="mC")
                m = sco.tile([P, F], f32, tag="m")
                nc.vector.tensor_tensor(mm2[:], s5[:, 0:2 * F], s5[:, 2 * F:4 * F], Alu.max)
                nc.vector.tensor_tensor(mC[:], mm2[:, 0:F], mm2[:, F:2 * F], Alu.max)
                nc.vector.tensor_tensor(m[:], mC[:], s_tiles[4], Alu.max)

                # ---------- phase 3: masks+counts, products, sums ----------
                acc = sm.tile([P, 16], f32, tag="acc")
                mask4 = msk.tile([P, 4 * F], mdt, tag="mask4")
                tots_c = ps.tile([P, 4], f32, tag="tots_c")
                cnts = sm.tile([P, 5], f32, tag="cnts")
                csum = sm.tile([P, 1], f32, tag="csum")
                recip = sm.tile([P, 5], f32, tag="recip")

                if PROD16:
                    # DVE emission is interleaved: non-accumulating products
                    # and the counts chain are spread between accumulating
                    # ops (masks, V-sums) to absorb accumulator-drain stalls
                    prod_fns = []  # (cluster, emit_fn), cluster-sorted
                    singles = {}
                    for d in range(3):
                        ks = [k for k in range(4) if SUMS[3 * k + d] == "S"]
                        if len(ks) == 4:
                            # two 2-wide pairs emitted post-mask-stream (keys
                            # 90+: never slotted between masks); (k0,k1) needs
                            # only mask0/1 so ScalarE's chain starts earlier
                            for ka in (0, 2):
                                def mk_pair2(d=d, ka=ka):
                                    pr = jnk.tile([P, 2 * F], mdt, tag=f"prq{ka}{d}")
                                    nc.vector.tensor_tensor(
                                        pr[:].rearrange("p (k f) -> p k f", k=2),
                                        mask4[:, ka * F:(ka + 2) * F]
                                            .rearrange("p (k f) -> p k f", k=2),
                                        pc16[:, d * F:(d + 1) * F]
                                            .rearrange("p (o f) -> p o f", o=1)
                                            .broadcast_to((P, 2, F)),
                                        Alu.mult)
                                    for i, k in enumerate((ka, ka + 1)):
                                        ja = jnk.tile([P, F], mdt, tag=f"ja{k}{d}")
                                        nc.scalar.activation(
                                            ja[:], pr[:, i * F:(i + 1) * F],
                                            Act.Identity,
                                            accum_out=acc[:, 4 + 3 * k + d:5 + 3 * k + d])
                                prod_fns.append((90 + ka, mk_pair2))
                        elif len(ks) == 3:
                            def mk_batch(d=d, ks=tuple(ks)):
                                pr = jnk.tile([P, 4 * F], mdt, tag=f"pr4{d}")
                                nc.vector.tensor_tensor(
                                    pr[:].rearrange("p (k f) -> p k f", k=4),
                                    mask4[:].rearrange("p (k f) -> p k f", k=4),
                                    pc16[:, d * F:(d + 1) * F]
                                        .rearrange("p (o f) -> p o f", o=1)
                                        .broadcast_to((P, 4, F)),
                                    Alu.mult)
                                for k in ks:
                                    ja = jnk.tile([P, F], mdt, tag=f"ja{k}{d}")
                                    nc.scalar.activation(
                                        ja[:], pr[:, k * F:(k + 1) * F],
                                        Act.Identity,
                                        accum_out=acc[:, 4 + 3 * k + d:5 + 3 * k + d])
                            prod_fns.append((3, mk_batch))
                        else:
                            for k in ks:
                                singles.setdefault(k, []).append(d)
                    for k, ds in singles.items():
                        if ds == [1, 2]:
                            # d1,d2 adjacent in channel-major pc16: one 2F TT
                            def mk_pair(k=k):
                                pr = jnk.tile([P, 2 * F], mdt, tag=f"prp{k}")
                                nc.vector.tensor_tensor(
                                    pr[:].rearrange("p (d f) -> p d f", d=2),
                                    mask4[:, k * F:(k + 1) * F]
                                        .rearrange("p (o f) -> p o f", o=1)
                                        .broadcast_to((P, 2, F)),
                                    pc16[:, F:3 * F].rearrange("p (d f) -> p d f", d=2),
                                    Alu.mult)
                                for i, d in enumerate((1, 2)):
                                    ja = jnk.tile([P, F], mdt, tag=f"ja{k}{d}")
                                    nc.scalar.activation(
                                        ja[:], pr[:, i * F:(i + 1) * F],
                                        Act.Identity,
                                        accum_out=acc[:, 4 + 3 * k + d:5 + 3 * k + d])
                            prod_fns.append((k, mk_pair))
                        else:
                            for d in ds:
                                def mk_single(d=d, k=k):
                                    pr = jnk.tile([P, F], mdt, tag=f"pr{k}{d}")
                                    nc.vector.tensor_tensor(
                                        pr[:], mask4[:, k * F:(k + 1) * F],
                                        pc16[:, d * F:(d + 1) * F], Alu.mult)
                                    ja = jnk.tile([P, F], mdt, tag=f"ja{k}{d}")
                                    nc.scalar.activation(
                                        ja[:], pr[:], Act.Identity,
                                        accum_out=acc[:, 4 + 3 * k + d:5 + 3 * k + d])
                                prod_fns.append((k, mk_single))
                    prod_fns.sort(key=lambda t: t[0])

                    # masks, with ready products slotted between them
                    for k in range(4):
                        nc.vector.scalar_tensor_tensor(
                            mask4[:, k * F:(k + 1) * F], s_tiles[k], 1.0, m[:],
                            Alu.mult, Alu.is_equal, accum_out=acc[:, k:k + 1])
                        if k < 3 and prod_fns and prod_fns[0][0] <= k:
                            prod_fns.pop(0)[1]()

                    # counts all-reduce as soon as the 4 mask accums land
                    nc.tensor.matmul(tots_c[:], ones128[:], acc[:, 0:4],
                                     start=True, stop=True)

                    # remaining products lead their V-sum (ScalarE needs them
                    # early); counts-chain ops trail theirs (their matmul
                    # input lands mid-stream)
                    pre = [f for _, f in prod_fns]
                    post = [
                        lambda: nc.vector.tensor_copy(cnts[:, 0:4], tots_c[:, 0:4]),
                        lambda: nc.vector.tensor_reduce(csum[:], tots_c[:, 0:4], X, Alu.add),
                        lambda: nc.vector.tensor_scalar(
                            cnts[:, 4:5], csum[:], -1.0, float(N), Alu.mult, Alu.add),
                        lambda: nc.vector.reciprocal(recip[:], cnts[:]),
                    ]
                    vsums = [(k, d) for k in range(4) for d in range(3)
                             if SUMS[3 * k + d] == "V"]
                    for i, (k, d) in enumerate(vsums):
                        if pre:
                            pre.pop(0)()
                        j = jnk.tile([P, F], mdt, tag=f"jv{k}{d}")
                        nc.vector.scalar_tensor_tensor(
                            j[:], pc16[:, d * F:(d + 1) * F], 1.0,
                            mask4[:, k * F:(k + 1) * F], Alu.mult,
                            Alu.mult, accum_out=acc[:, 4 + 3 * k + d:5 + 3 * k + d])
                        if not pre and post:
                            post.pop(0)()
                    for f in pre + post:
                        f()
                else:
                    for k in range(4):
                        nc.vector.scalar_tensor_tensor(
                            mask4[:, k * F:(k + 1) * F], s_tiles[k], 1.0, m[:],
                            Alu.mult, Alu.is_equal, accum_out=acc[:, k:k + 1])
                    nc.tensor.matmul(tots_c[:], ones128[:], acc[:, 0:4],
                                     start=True, stop=True)
                    nc.vector.tensor_copy(cnts[:, 0:4], tots_c[:, 0:4])
                    nc.vector.tensor_reduce(csum[:], tots_c[:, 0:4], X, Alu.add)
                    nc.vector.tensor_scalar(cnts[:, 4:5], csum[:], -1.0, float(N),
                                            Alu.mult, Alu.add)
                    nc.vector.reciprocal(recip[:], cnts[:])
                    for k in range(4):
                        for d in range(3):
                            col = acc[:, 4 + 3 * k + d:5 + 3 * k + d]
                            pcd = pc[:, d * F:(d + 1) * F]
                            if SUMS[3 * k + d] == "S":
                                pr = jnk.tile([P, F], f32, tag=f"pr{k}{d}")
                                nc.vector.tensor_tensor(
                                    pr[:], mask4[:, k * F:(k + 1) * F], pcd, Alu.mult)
                                ja = jnk.tile([P, F], f32, tag=f"ja{k}{d}")
                                nc.scalar.activation(
                                    ja[:], pr[:], Act.Identity, accum_out=col)
                            else:
                                j = jnk.tile([P, F], f32, tag=f"jv{k}{d}")
                                nc.vector.scalar_tensor_tensor(
                                    j[:], pcd, 1.0, mask4[:, k * F:(k + 1) * F],
                                    Alu.mult, Alu.mult, accum_out=col)

                # ---------- tail (sums all-reduce + center update) ----------
                tots = ps.tile([P, 12], f32, tag="tots")
                nc.tensor.matmul(tots[:], ones128[:], acc[:, 4:16], start=True, stop=True)

                # clusters 0..3 first (centers then biases) so next-iter u_0..3
                # ACTs unblock while the cluster-4 chain is still running
                cb = sm.tile([P, 20], f32, tag="cb")
                sq = sm.tile([P, 15], f32, tag="sq")
                c2 = sm.tile([P, 5], f32, tag="c2")
                nc.vector.tensor_tensor(
                    cb[:, 0:12].rearrange("p (k d) -> p k d", d=3),
                    tots[:, 0:12].rearrange("p (k d) -> p k d", d=3),
                    recip[:, 0:4].rearrange("p (k o) -> p k o", o=1).broadcast_to((P, 4, 3)),
                    Alu.mult)
                nc.vector.tensor_tensor(sq[:, 0:12], cb[:, 0:12], cb[:, 0:12], Alu.mult)
                nc.vector.tensor_reduce(
                    c2[:, 0:4], sq[:, 0:12].rearrange("p (k d) -> p k d", d=3), X, Alu.add)
                nc.vector.tensor_scalar(cb[:, 15:19], c2[:, 0:4], -0.5, 2.0,
                                        Alu.mult, Alu.add)
                s4p = sm.tile([P, 3], f32, tag="s4p")
                nc.vector.tensor_reduce(
                    s4p[:], tots[:, 0:12].rearrange("p (k d) -> p d k", d=3), X, Alu.add)
                S4 = sm.tile([P, 3], f32, tag="S4")
                nc.vector.tensor_tensor(S4[:], tot3[:], s4p[:], Alu.subtract)
                nc.vector.tensor_tensor(
                    cb[:, 12:15],
                    S4[:], recip[:, 4:5].broadcast_to((P, 3)), Alu.mult)
                nc.vector.tensor_tensor(sq[:, 12:15], cb[:, 12:15], cb[:, 12:15], Alu.mult)
                nc.vector.tensor_reduce(
                    c2[:, 4:5], sq[:, 12:15].rearrange("p (k d) -> p k d", d=3), X, Alu.add)
                nc.vector.tensor_scalar(cb[:, 19:20], c2[:, 4:5], -0.5, 2.0,
                                        Alu.mult, Alu.add)

                prev_traj = (cnts, cb, t)
                rep = cb

            # flush final trajectory snapshots
            pcn, pcb, pt = prev_traj
            nc.scalar.copy(counts_st[0:1, 5 * (pt - 1):5 * pt], pcn[0:1, :])
            nc.scalar.copy(cent_st[0:1, 15 * pt:15 * (pt + 1)], pcb[0:1, 0:15])

            nc.sync.dma_start(out=outv[0:1, 0:500], in_=counts_st[:])
            nc.sync.dma_start(out=outv[0:1, 500:OUT_LEN], in_=cent_st[:])
    nc.compile()
    return nc


def _get_nc():
    if "nc" not in _CACHE:
        _CACHE["nc"] = _build_nc()
    return _CACHE["nc"]


def _host_finalize(counts_all, cent_all):
    B = counts_all.shape[0]
    prev = cent_all[:, :-1, :]
    new = cent_all[:, 1:, :]
    with np.errstate(invalid="ignore"):
        ok = np.abs(prev - new) <= np.float32(ATOL) + np.float32(RTOL) * np.abs(new)
    conv_t = np.all(ok, axis=(0, 2))
    idx = np.nonzero(conv_t)[0]
    T = int(idx[0]) + 1 if len(idx) else ITERS + 1
    L = min(T, ITERS)
    centers = cent_all[:, T - 1].reshape(B, K, 3)
    percentages = counts_all[:, L - 1] / np.float32(N)
    centers = np.clip(centers, 0.0, 1.0)
    percentages = np.clip(percentages, 0.0, 1.0)
    color_info = np.concatenate([centers, percentages[..., None]], axis=2).astype(np.float32)
    color_info = np.nan_to_num(color_info, nan=0.0, posinf=1.0, neginf=0.0)
    sort_idx = np.argsort(-color_info[:, :, 3], axis=1, kind="stable")
    return color_info[sort_idx]


def _make_inputs(x, init_idx):
    B = x.shape[0]
    x = np.ascontiguousarray(np.asarray(x, dtype=np.float32))
    init_idx = np.asarray(init_idx).astype(np.int64)
    hh, ww = init_idx // 224, init_idx % 224
    in_maps = []
    for b in range(B):
        c0 = (x[b, :, hh, ww] + np.float32(1e-8)).astype(np.float32)
        cb0 = np.zeros((1, 20), np.float32)
        cb0[0, :15] = c0.reshape(15)
        c2 = (c0 * c0).sum(axis=1, dtype=np.float32)
        cb0[0, 15:20] = np.float32(2.0) - np.float32(0.5) * c2
        in_maps.append({"xp": x[b].reshape(3, N), "cbin": cb0})
    return in_maps


def kernel(x, init_idx):
    from concourse.bass_utils import run_bass_kernel_spmd

    nc = _get_nc()
    in_maps = _make_inputs(x, init_idx)
    res = run_bass_kernel_spmd(nc, in_maps, list(range(8)))
    outs = [np.asarray(r["outv"]).reshape(OUT_LEN) for r in res.results]
    counts_all = np.stack([o[0:500].reshape(100, 5) for o in outs])
    cent_all = np.stack([o[500:OUT_LEN].reshape(101, 15) for o in outs])
    return _host_finalize(counts_all, cent_all)
